# revision 1
# baseline (speedup 1.0000x reference)
"""CapsuleLayer (dynamic routing) Trainium2 kernel.

Problem: x [64, 2048, 8], W [1, 2048, 32, 16, 8] (f32)
  u_hat[b,i,o,j] = sum_d W[0,i,o,j,d] * x[b,i,d]
  3 routing iterations (softmax over o, weighted sum over i, squash,
  logit update), returns v [64, 32, 16].

Strategy: data-parallel over batch across 8 NeuronCores (8 samples per
core), W replicated. Per core, u_hat for a 4-sample chunk stays resident
in SBUF ([128 partitions (i%128), 4b x 16it x 512oj] f32 = 128KB/part),
so the routing loop never touches HBM. u_hat is built with fused
scalar_tensor_tensor MACs (per-partition scalar = x[b,i,d]) split across
VectorE and GpSimd while W streams in per i-tile. The i-reduction
(s_j = sum_i c*u_hat) runs on the TensorEngine as K=128 matmuls
accumulating in PSUM, with a mask+strided-reduce extracting the diagonal
o-blocks. Logit updates (sum_j u_hat*v) run as DVE mul + strided reduce
with v broadcast to 128 partitions via a K=1 PE matmul.
"""

import sys

import numpy as np

sys.path.insert(0, "/opt/trn_rl_repo")

import concourse.bacc as bacc
import concourse.mybir as mybir
from concourse import bass_utils
from concourse.tile import TileContext

F32 = mybir.dt.float32

N_CORES = 8
B, IN_CAPS, IN_DIM, OUT_CAPS, OUT_DIM = 64, 2048, 8, 32, 16
BC = B // N_CORES            # samples per core
CHUNK = 4                    # samples whose u_hat is SBUF-resident at once
N_IT = IN_CAPS // 128        # i-tiles of 128 partitions
OJ = OUT_CAPS * OUT_DIM      # flattened (o, j) = 512
NUM_ROUTING = 3
EPS = 1e-9

_CACHE: dict = {}


def build_nc(n_routing=NUM_ROUTING, do_update=True, use_gp=True):
    mult = mybir.AluOpType.mult
    add = mybir.AluOpType.add
    AX = mybir.AxisListType.X
    Exp = mybir.ActivationFunctionType.Exp
    Sqrt = mybir.ActivationFunctionType.Sqrt

    nc = bacc.Bacc(
        "TRN2",
        target_bir_lowering=False,
        debug=False,
        enable_asserts=False,
        num_devices=1,
    )
    xs_d = nc.dram_tensor("xs", [128, BC, N_IT, IN_DIM], F32, kind="ExternalInput")
    wt_d = nc.dram_tensor("wt", [N_IT, 128, IN_DIM, OJ], F32, kind="ExternalInput")
    mask_d = nc.dram_tensor("mask", [OUT_CAPS, OJ], F32, kind="ExternalInput")
    ones_d = nc.dram_tensor("ones", [1, 128], F32, kind="ExternalInput")
    out_d = nc.dram_tensor("vout", [BC, OUT_CAPS, OUT_DIM], F32, kind="ExternalOutput")

    with TileContext(nc) as tc:
        with (
            tc.tile_pool(name="per", bufs=1) as per,
            tc.tile_pool(name="wpool", bufs=2) as wpool,
            tc.tile_pool(name="sm", bufs=3) as sm,
            tc.tile_pool(name="pb", bufs=2) as pb,
            tc.tile_pool(name="gp", bufs=2, space="PSUM") as gp,
            tc.tile_pool(name="vp", bufs=2, space="PSUM") as vp,
        ):
            xs = per.tile([128, BC, N_IT, IN_DIM], F32, tag="xs")
            nc.sync.dma_start(xs[:], xs_d.ap())
            mask = per.tile([OUT_CAPS, OJ], F32, tag="mask")
            nc.sync.dma_start(mask[:], mask_d.ap())
            ones = per.tile([1, 128], F32, tag="ones")
            nc.sync.dma_start(ones[:], ones_d.ap())
            zb = per.tile([OUT_CAPS, 1], F32, tag="zb")
            nc.vector.memset(zb[:], 0.0)

            uhat = per.tile([128, CHUNK, N_IT, OJ], F32, tag="uhat")
            bij = per.tile([128, CHUNK, N_IT, OUT_CAPS], F32, tag="bij")
            chat = per.tile([128, CHUNK, N_IT, OUT_CAPS], F32, tag="chat")
            zred = per.tile([128, CHUNK, N_IT], F32, tag="zred")

            for chunk in range(BC // CHUNK):
                # ---- build u_hat for this chunk ----
                for it in range(N_IT):
                    w = wpool.tile([128, IN_DIM, OJ], F32, tag="w")
                    nc.sync.dma_start(w[:], wt_d.ap()[it])
                    for b in range(CHUNK):
                        gb = chunk * CHUNK + b
                        uh = uhat[:, b, it, :]
                        nc.vector.tensor_scalar_mul(
                            uh, w[:, 0, :], xs[:, gb, it, 0:1]
                        )
                        for d in range(1, IN_DIM):
                            eng = nc.vector
                            eng.scalar_tensor_tensor(
                                uh, w[:, d, :], xs[:, gb, it, d : d + 1], uh,
                                op0=mult, op1=add,
                            )
                (nc.gpsimd if use_gp else nc.vector).memset(bij[:], 0.0)

                # ---- routing ----
                for r in range(n_routing):
                    if r == 0:
                        (nc.gpsimd if use_gp else nc.vector).memset(
                            chat[:], 1.0 / OUT_CAPS
                        )
                    else:
                        nc.scalar.activation(chat[:], bij[:], Exp)
                        nc.vector.tensor_reduce(zred[:], chat[:], axis=AX, op=add)
                        nc.vector.reciprocal(zred[:], zred[:])
                        for b in range(CHUNK):
                            for it in range(N_IT):
                                nc.vector.tensor_scalar_mul(
                                    chat[:, b, it, :],
                                    chat[:, b, it, :],
                                    zred[:, b, it : it + 1],
                                )
                    for b in range(CHUNK):
                        g = gp.tile([OUT_CAPS, OJ], F32, tag="g")
                        for it in range(N_IT):
                            nc.tensor.matmul(
                                g[:],
                                chat[:, b, it, :],
                                uhat[:, b, it, :],
                                start=(it == 0),
                                stop=(it == N_IT - 1),
                            )
                        mprod = sm.tile([OUT_CAPS, OJ], F32, tag="mprod")
                        nc.vector.tensor_mul(mprod[:], g[:], mask[:])
                        s_b = sm.tile([OUT_CAPS, OUT_DIM], F32, tag="s_b")
                        nc.vector.tensor_reduce(
                            s_b[:],
                            mprod[:].rearrange("p (o j) -> p j o", o=OUT_CAPS),
                            axis=AX,
                            op=add,
                        )
                        # squash: v = s * s2 / ((1+s2) * sqrt(s2+eps))
                        sq = sm.tile([OUT_CAPS, OUT_DIM], F32, tag="sq")
                        s2 = sm.tile([OUT_CAPS, 1], F32, tag="s2")
                        nc.vector.tensor_mul(sq[:], s_b[:], s_b[:])
                        nc.vector.tensor_reduce(s2[:], sq[:], axis=AX, op=add)
                        t1 = sm.tile([OUT_CAPS, 1], F32, tag="t1")
                        nc.vector.tensor_scalar_add(t1[:], s2[:], 1.0)
                        nc.vector.reciprocal(t1[:], t1[:])
                        t2 = sm.tile([OUT_CAPS, 1], F32, tag="t2")
                        nc.vector.tensor_scalar_add(t2[:], s2[:], EPS)
                        nc.scalar.activation(t2[:], t2[:], Sqrt, bias=zb[:])
                        nc.vector.reciprocal(t2[:], t2[:])
                        nc.vector.tensor_mul(t1[:], t1[:], t2[:])
                        nc.vector.tensor_mul(t1[:], t1[:], s2[:])
                        v_b = sm.tile([OUT_CAPS, OUT_DIM], F32, tag="v_b")
                        nc.vector.tensor_scalar_mul(v_b[:], s_b[:], t1[:])
                        if r < n_routing - 1 and do_update:
                            vrow = sm.tile([1, OJ], F32, tag="vrow")
                            nc.sync.dma_start(vrow[:], v_b[:])
                            vps = vp.tile([128, OJ], F32, tag="vps")
                            nc.tensor.matmul(
                                vps[:], ones[:], vrow[:], start=True, stop=True
                            )
                            vsb = pb.tile([128, OJ], F32, tag="vsb")
                            nc.scalar.copy(vsb[:], vps[:])
                            vv = vsb[:].rearrange("p (o j) -> p o j", o=OUT_CAPS)
                            for it in range(N_IT):
                                prod = pb.tile(
                                    [128, OUT_CAPS, OUT_DIM], F32, tag="prod"
                                )
                                uu = uhat[:, b, it, :].rearrange(
                                    "p (o j) -> p o j", o=OUT_CAPS
                                )
                                eng = (
                                    nc.vector
                                    if (it % 2 == 0 or not use_gp)
                                    else nc.gpsimd
                                )
                                eng.tensor_mul(prod[:], uu, vv)
                                upd = sm.tile([128, OUT_CAPS], F32, tag="upd")
                                nc.vector.tensor_reduce(
                                    upd[:], prod[:], axis=AX, op=add
                                )
                                nc.vector.tensor_add(
                                    bij[:, b, it, :], bij[:, b, it, :], upd[:]
                                )
                        else:
                            nc.sync.dma_start(
                                out_d.ap()[chunk * CHUNK + b], v_b[:]
                            )
    nc.compile()
    return nc


def _prep_inputs(x: np.ndarray, W: np.ndarray):
    # Per-core x shard: xs[p, b, it, d] = x[core*BC + b, it*128 + p, d]
    W0 = np.ascontiguousarray(W.reshape(IN_CAPS, OUT_CAPS, OUT_DIM, IN_DIM))
    wt = np.ascontiguousarray(
        W0.reshape(N_IT, 128, OUT_CAPS, OUT_DIM, IN_DIM)
        .transpose(0, 1, 4, 2, 3)
        .reshape(N_IT, 128, IN_DIM, OJ)
    ).astype(np.float32)
    mask = np.zeros((OUT_CAPS, OJ), np.float32)
    for o in range(OUT_CAPS):
        mask[o, o * OUT_DIM : (o + 1) * OUT_DIM] = 1.0
    ones = np.ones((1, 128), np.float32)
    in_maps = []
    for c in range(N_CORES):
        xc = x[c * BC : (c + 1) * BC]  # [BC, 2048, 8]
        xs = np.ascontiguousarray(
            xc.reshape(BC, N_IT, 128, IN_DIM).transpose(2, 0, 1, 3)
        ).astype(np.float32)
        in_maps.append({"xs": xs, "wt": wt, "mask": mask, "ones": ones})
    return in_maps


def kernel(x: np.ndarray, W: np.ndarray) -> np.ndarray:
    x = np.asarray(x, dtype=np.float32)
    W = np.asarray(W, dtype=np.float32)
    if "nc" not in _CACHE:
        _CACHE["nc"] = build_nc()
    nc = _CACHE["nc"]
    in_maps = _prep_inputs(x, W)
    res = bass_utils.run_bass_kernel_spmd(nc, in_maps, core_ids=list(range(N_CORES)))
    out = np.concatenate([res.results[c]["vout"] for c in range(N_CORES)], axis=0)
    return out.astype(np.float32)


if __name__ == "__main__":
    xt = np.random.randn(B, IN_CAPS, IN_DIM).astype(np.float32)
    Wt = (np.random.randn(1, IN_CAPS, OUT_CAPS, OUT_DIM, IN_DIM) * 0.01).astype(
        np.float32
    )
    print(kernel(xt, Wt).shape)



# revision 13
# speedup vs baseline: 1.8711x; 1.8711x over previous
"""CapsuleLayer (dynamic routing) Trainium2 kernel, v2.

Problem: x [64, 2048, 8], W [1, 2048, 32, 16, 8] (f32)
  u_hat[b,i,o,j] = sum_d W[0,i,o,j,d] * x[b,i,d]
  3 routing iterations (softmax over o, weighted i-sum, squash, logit
  update), returns v [64, 32, 16].

Strategy: data-parallel over batch across 8 NeuronCores (BC=8 samples
per core), W replicated (bf16).

Per core, u_hat is built on the TensorEngine in ONE pass over W:
for each block t of 16 input capsules, a single matmul with
  lhsT = xdiag[t]  [K=(s16,d8)=128, M=(b8,s16)=128]   (block-diagonal x)
  rhs  = w2[t]     [K=128, N=(o,j)=512]
yields u_hat for all 8 samples x 16 capsules x 512 (o,j) in one
instruction (the delta_{s,s'} structure carries the capsule index
through the contraction). PSUM is evacuated to a resident bf16
u_hat [128=(b,s), t, (o,j)] by alternating ScalarE/VectorE copies.

Routing sums s_j run on the TensorEngine as K=64 matmuls per half
(partitions (b0-3,s) / (b4-7,s)) with block-diagonal softmax(c)
stationaries, accumulating over t in PSUM; r0 is fused into the build
pass as a second accumulation chain with lhsT = x/32. Logit updates
(sum_j u_hat*v) are bf16 mul (VectorE/GpSimd) + strided reduce.
"""

import sys

import numpy as np
import ml_dtypes

sys.path.insert(0, "/opt/trn_rl_repo")

import concourse.bacc as bacc
import concourse.mybir as mybir
from concourse import bass_utils
from concourse.tile import TileContext

F32 = mybir.dt.float32
BF16 = mybir.dt.bfloat16
NPBF16 = ml_dtypes.bfloat16

N_CORES = 8
B, IN_CAPS, IN_DIM, OUT_CAPS, OUT_DIM = 64, 2048, 8, 32, 16
BC = B // N_CORES            # samples per core
S = 16                       # capsules per i-block
T = IN_CAPS // S             # 128 i-blocks
D = IN_DIM
OJ = OUT_CAPS * OUT_DIM      # 512
NUM_ROUTING = 3
EPS = 1e-9
TC = 4                       # t-chunk for the update/softmax pipeline

_CACHE: dict = {}


def build_nc():
    mult = mybir.AluOpType.mult
    add = mybir.AluOpType.add
    AX = mybir.AxisListType.X
    Exp = mybir.ActivationFunctionType.Exp
    Sqrt = mybir.ActivationFunctionType.Sqrt

    nc = bacc.Bacc(
        "TRN2",
        target_bir_lowering=False,
        debug=False,
        enable_asserts=False,
        num_devices=1,
    )
    w2_d = nc.dram_tensor("w2", [T, 128, OJ], BF16, kind="ExternalInput")
    xdiag_d = nc.dram_tensor("xdiag", [T, 128, 128], BF16, kind="ExternalInput")
    xs0_d = nc.dram_tensor("xs0", [128, T, BC], BF16, kind="ExternalInput")
    mask_d = nc.dram_tensor("mask", [128, OJ], BF16, kind="ExternalInput")
    bmat_d = nc.dram_tensor("bmat", [BC, 128], BF16, kind="ExternalInput")
    out_d = nc.dram_tensor("vout", [BC, OUT_CAPS, OUT_DIM], F32, kind="ExternalOutput")

    with TileContext(nc) as tc:
        with (
            tc.tile_pool(name="per", bufs=1) as per,
            tc.tile_pool(name="wp", bufs=3) as wp,
            tc.tile_pool(name="xp", bufs=4) as xp,
            tc.tile_pool(name="sm", bufs=1) as sm,
            tc.tile_pool(name="pp", bufs=2) as pp,
            tc.tile_pool(name="up", bufs=2) as up,
            tc.tile_pool(name="vb", bufs=1) as vb,
            tc.tile_pool(name="bp", bufs=4, space="PSUM") as bp,
            tc.tile_pool(name="s0p", bufs=1, space="PSUM") as s0pool,
            tc.tile_pool(name="sjp", bufs=1, space="PSUM") as sjpool,
            tc.tile_pool(name="vp", bufs=1, space="PSUM") as vp,
        ):
            # ---- persistent tiles ----
            uhat = per.tile([128, T, OJ], BF16, tag="uhat")
            bij = per.tile([128, T, OUT_CAPS], F32, tag="bij")
            chat = per.tile([128, T, OUT_CAPS], BF16, tag="chat")
            biglhs = per.tile([128, T, 128], BF16, tag="biglhs")
            xs0s = per.tile([128, T, BC], BF16, tag="xs0s")
            masks = per.tile([128, OJ], BF16, tag="masks")
            bmats = per.tile([BC, 128], BF16, tag="bmats")
            zred = per.tile([128, T], F32, tag="zred")
            zb = per.tile([128, 1], F32, tag="zb")
            nc.vector.memset(zb[:], 0.0)

            nc.sync.dma_start(xs0s[:], xs0_d.ap())
            nc.sync.dma_start(masks[:], mask_d.ap())
            nc.sync.dma_start(bmats[:], bmat_d.ap())
            nc.gpsimd.memset(biglhs[:], 0.0)
            nc.gpsimd.memset(bij[:], 0.0)

            # ---- phase A: build u_hat (1 matmul / 16 capsules) + s0 ----
            s0p = s0pool.tile([BC, OJ], F32, tag="s0")
            for t in range(T):
                w = wp.tile([128, OJ], BF16, tag="w")
                nc.sync.dma_start(w[:], w2_d.ap()[t])
                xd = xp.tile([128, 128], BF16, tag="xd")
                nc.sync.dma_start(xd[:], xdiag_d.ap()[t])
                ub = bp.tile([128, OJ], F32, tag="ub")
                nc.tensor.matmul(ub[:], xd[:], w[:], start=True, stop=True)
                nc.tensor.matmul(
                    s0p[:], xs0s[:, t, :], w[:],
                    start=(t == 0), stop=(t == T - 1),
                )
                eng = nc.scalar if t % 2 == 0 else nc.vector
                if t % 2 == 0:
                    eng.copy(uhat[:, t, :], ub[:])
                else:
                    eng.tensor_copy(uhat[:, t, :], ub[:])

            # ---- r0: v0 = squash(s0), broadcast to all partitions ----
            # s0p [BC, (o, j)] f32 (1/32 pre-folded into xs0).
            sq8f = sm.tile([128, OJ], F32, tag="mp")
            sq8 = sq8f[0:BC, :]
            nc.scalar.activation(
                sq8, s0p[:], mybir.ActivationFunctionType.Square,
                bias=zb[0:BC, :],
            )
            s28 = sm.tile([BC, OUT_CAPS], F32, tag="s28")
            nc.vector.tensor_reduce(
                s28[:], sq8.rearrange("p (o j) -> p o j", o=OUT_CAPS),
                axis=AX, op=add,
            )
            t1 = sm.tile([BC, OUT_CAPS], F32, tag="t1a")
            nc.vector.tensor_scalar_add(t1[:], s28[:], 1.0)
            nc.vector.reciprocal(t1[:], t1[:])
            t2 = sm.tile([BC, OUT_CAPS], F32, tag="t2a")
            nc.vector.tensor_scalar_add(t2[:], s28[:], EPS)
            nc.scalar.activation(t2[:], t2[:], Sqrt, bias=zb[0:BC, :])
            nc.vector.reciprocal(t2[:], t2[:])
            nc.vector.tensor_mul(t1[:], t1[:], t2[:])
            nc.vector.tensor_mul(t1[:], t1[:], s28[:])
            v0 = sm.tile([BC, OJ], BF16, tag="v0")
            # v = s * t1, broadcasting t1 over j via a (j, o)-strided view.
            nc.vector.tensor_mul(
                v0[:].rearrange("p (o j) -> p j o", o=OUT_CAPS),
                s0p[:].rearrange("p (o j) -> p j o", o=OUT_CAPS),
                t1[:].unsqueeze(1).to_broadcast([BC, OUT_DIM, OUT_CAPS]),
            )
            vbp = vp.tile([128, OJ], F32, tag="vbp")
            nc.tensor.matmul(vbp[:], bmats[:], v0[:], start=True, stop=True)
            vbc = vb.tile([128, OJ], BF16, tag="vbc")
            nc.scalar.copy(vbc[:], vbp[:])

            # ---- routing iterations ----
            for r in (1, 2):
                sjA = sjpool.tile([128, OJ], F32, tag="sjA")
                sjB = sjpool.tile([128, OJ], F32, tag="sjB")
                for ci in range(T // TC):
                    ts = slice(ci * TC, (ci + 1) * TC)
                    # logit update for chunk: bij += sum_j u_hat * v
                    prod = pp.tile([128, TC, OJ], BF16, tag="prod")
                    meng = nc.gpsimd if ci % 3 != 2 else nc.vector
                    meng.tensor_mul(
                        prod[:], uhat[:, ts, :],
                        vbc[:].unsqueeze(1).to_broadcast([128, TC, OJ]),
                    )
                    updc = up.tile([128, TC, OUT_CAPS], F32, tag="updc")
                    nc.vector.tensor_reduce(
                        updc[:],
                        prod[:].rearrange("p t (o j) -> p t o j", o=OUT_CAPS),
                        axis=AX, op=add,
                    )
                    nc.vector.tensor_add(bij[:, ts, :], bij[:, ts, :], updc[:])
                    # softmax over o for chunk
                    nc.scalar.activation(chat[:, ts, :], bij[:, ts, :], Exp)
                    nc.vector.tensor_reduce(
                        zred[:, ts], chat[:, ts, :], axis=AX, op=add
                    )
                    nc.vector.reciprocal(zred[:, ts], zred[:, ts])
                    nc.vector.tensor_mul(
                        chat[:, ts, :], chat[:, ts, :],
                        zred[:, ts].unsqueeze(2).to_broadcast(
                            [128, TC, OUT_CAPS]
                        ),
                    )
                    # place c into block-diagonal stationaries (odd b's
                    # start at partition 16 mod 32, which engine APs can't
                    # address -> route those through SBUF-to-SBUF DMA)
                    for b in range(BC):
                        dst = biglhs[
                            b * S : (b + 1) * S, ts,
                            (b % 4) * OUT_CAPS : (b % 4 + 1) * OUT_CAPS,
                        ]
                        src = chat[b * S : (b + 1) * S, ts, :]
                        if b % 2 == 0:
                            nc.vector.tensor_copy(dst, src)
                        else:
                            nc.sync.dma_start(dst, src)
                    # s_j partial sums for chunk
                    for t in range(ci * TC, (ci + 1) * TC):
                        nc.tensor.matmul(
                            sjA[:], biglhs[0:64, t, :], uhat[0:64, t, :],
                            start=(t == 0), stop=(t == T - 1),
                        )
                        nc.tensor.matmul(
                            sjB[:], biglhs[64:128, t, :], uhat[64:128, t, :],
                            start=(t == 0), stop=(t == T - 1),
                        )
                # extract diagonal blocks + squash, per half
                vhalves = []
                for sj in (sjA, sjB):
                    mp = sm.tile([128, OJ], F32, tag="mp")
                    nc.vector.tensor_mul(mp[:], sj[:], masks[:])
                    s_h = sm.tile([128, OUT_DIM], F32, tag="s_h")
                    nc.vector.tensor_reduce(
                        s_h[:],
                        mp[:].rearrange("p (o j) -> p j o", o=OUT_CAPS),
                        axis=AX, op=add,
                    )
                    sq = sm.tile([128, OUT_DIM], F32, tag="sq")
                    nc.vector.tensor_mul(sq[:], s_h[:], s_h[:])
                    s2 = sm.tile([128, 1], F32, tag="s2")
                    nc.vector.tensor_reduce(s2[:], sq[:], axis=AX, op=add)
                    ta = sm.tile([128, 1], F32, tag="ta")
                    nc.vector.tensor_scalar_add(ta[:], s2[:], 1.0)
                    nc.vector.reciprocal(ta[:], ta[:])
                    tb = sm.tile([128, 1], F32, tag="tb")
                    nc.vector.tensor_scalar_add(tb[:], s2[:], EPS)
                    nc.scalar.activation(tb[:], tb[:], Sqrt, bias=zb[:])
                    nc.vector.reciprocal(tb[:], tb[:])
                    nc.vector.tensor_mul(ta[:], ta[:], tb[:])
                    nc.vector.tensor_mul(ta[:], ta[:], s2[:])
                    vh = sm.tile(
                        [128, OUT_DIM], BF16 if r < 2 else F32, tag="vh"
                    )
                    nc.vector.tensor_scalar_mul(vh[:], s_h[:], ta[:])
                    vhalves.append(vh)
                if r < 2:
                    vcomp = sm.tile([BC, OJ], BF16, tag="vcomp")
                    nc.sync.dma_start(vcomp[0:4, :], vhalves[0][:])
                    nc.sync.dma_start(vcomp[4:8, :], vhalves[1][:])
                    vbp = vp.tile([128, OJ], F32, tag="vbp")
                    nc.tensor.matmul(
                        vbp[:], bmats[:], vcomp[:], start=True, stop=True
                    )
                    vbc = vb.tile([128, OJ], BF16, tag="vbc")
                    nc.scalar.copy(vbc[:], vbp[:])
                else:
                    nc.sync.dma_start(out_d.ap()[0:4], vhalves[0][:])
                    nc.sync.dma_start(out_d.ap()[4:8], vhalves[1][:])
    nc.compile()
    return nc


def _prep_inputs(x: np.ndarray, W: np.ndarray):
    # w2[t, s*8+d, o*16+j] = W[0, 16t+s, o, j, d]
    W0 = W.reshape(IN_CAPS, OUT_CAPS, OUT_DIM, IN_DIM)
    w2 = np.ascontiguousarray(
        W0.reshape(T, S, OUT_CAPS, OUT_DIM, IN_DIM)
        .transpose(0, 1, 4, 2, 3)
        .reshape(T, 128, OJ)
    ).astype(NPBF16)
    # mask[p, o'*16+j] = (o' == p % 32)
    p = np.arange(128)[:, None]
    op = np.arange(OJ)[None, :] // OUT_DIM
    mask = (op == (p % OUT_CAPS)).astype(NPBF16)
    # bmat[b, b'*16+s] = (b == b')
    bmat = (np.arange(128)[None, :] // S == np.arange(BC)[:, None]).astype(
        NPBF16
    )
    in_maps = []
    s_idx = np.arange(S)
    for c in range(N_CORES):
        xc = x[c * BC : (c + 1) * BC]  # [BC, 2048, 8]
        xr = np.ascontiguousarray(
            xc.reshape(BC, T, S, D).transpose(1, 2, 3, 0)
        )  # [t, s, d, b]
        # xdiag[t, s*8+d, b*16+s'] = xr[t, s, d, b] * (s == s')
        arr = np.zeros((T, S, D, BC, S), np.float32)
        arr[:, s_idx, :, :, s_idx] = xr.transpose(1, 0, 2, 3)
        xdiag = arr.reshape(T, 128, 128).astype(NPBF16)
        # xs0[s*8+d, t, b] = xr[t, s, d, b] / 32
        xs0 = (
            np.ascontiguousarray(xr.transpose(1, 2, 0, 3)).reshape(128, T, BC)
            / np.float32(OUT_CAPS)
        ).astype(NPBF16)
        in_maps.append(
            {"w2": w2, "xdiag": xdiag, "xs0": xs0, "mask": mask, "bmat": bmat}
        )
    return in_maps


def kernel(x: np.ndarray, W: np.ndarray) -> np.ndarray:
    x = np.asarray(x, dtype=np.float32)
    W = np.asarray(W, dtype=np.float32)
    if "nc" not in _CACHE:
        _CACHE["nc"] = build_nc()
    nc = _CACHE["nc"]
    in_maps = _prep_inputs(x, W)
    res = bass_utils.run_bass_kernel_spmd(nc, in_maps, core_ids=list(range(N_CORES)))
    out = np.concatenate([res.results[c]["vout"] for c in range(N_CORES)], axis=0)
    return out.astype(np.float32)


if __name__ == "__main__":
    xt = np.random.randn(B, IN_CAPS, IN_DIM).astype(np.float32)
    Wt = (np.random.randn(1, IN_CAPS, OUT_CAPS, OUT_DIM, IN_DIM) * 0.01).astype(
        np.float32
    )
    print(kernel(xt, Wt).shape)
